# revision 9
# baseline (speedup 1.0000x reference)
"""Weighted 2D cross-entropy (BCE-over-classes) loss on 8 Trainium2 cores.

Math (matches the reference):
  t in [0,19); pos = t>0, neg = t==0 (all pixels are pos or neg; mask == 1)
  S(i) = sum_c bce(i,c) = -lnR(i)
     lnR(i) = A(i) + B(i)
     A(i)   = sum_c ln(1-p_c(i))
     B(i)   = ln(p_t(i)) - ln(1-p_t(i)) = ln(e^{-lsel(i)} - 1),  lsel = ln(1-p_t)
  loss = ( (NEG/TOT)*S_pos_sum + (POS/TOT)*S_neg_sum ) / (TOT*C)

Per-core (core k <- batch element k, pure data parallel), FOUR quarter-passes
over pixel quarters [128, 1024].  A quarter's PSUM accumulators (A + lsel)
occupy 4 banks, so two quarters ping-pong in PSUM: while quarter q's tail
(Exp/Ln/STT chain on ACT+DVE) drains its PSUM banks, the PE already streams
quarter q+1's matmuls -- no half-boundary stall.
Classes are processed in PAIRS per quarter: one 3D DMA brings both classes'
quarter into a [128, 2, 1024] tile, so ACT runs ONE 2048-wide Ln and DVE ONE
2048-wide mult per pair -- the ~294ns fixed per-activation overhead halves
versus 1024-wide ops (ACT is the pipeline pacer once DMA stalls are gone).
  - per pair: 1MB DMA of p, ACT Ln(1-p)->bf16, DVE eq=(t==c) (4x, x2) and
    masked=eq*L (2x), PE identity-matmuls accumulate A and lsel in PSUM.
  - tail per quarter: expm=Exp(-lsel); B=Ln(expm-1) (fused -1 bias via a
    [128,1] const column, no DVE subtract); lnR=B+A via STT with accum_out;
    pos-masked sum via a second STT accum.
  - the unpaired class 18 runs per-quarter; in the last quarter it and the
    tail run in 512-wide chunks so the post-last-DMA drain is short.
Target is converted to bf16 on HOST (1MB instead of 2MB int32 DMA, no
on-chip CAST, and the first predict tile lands sooner).
Activation tables are pinned to natural_log_exp_and_others (holds both
ln and exp) -- otherwise bacc's table-load pass alternates between the
ln-only and exp-only sets, paying ~1.3us per reload.
Counts (pos/neg) are computed on host from the int target directly.
Per-core output is the raw [128, 16] per-partition stats; the final
partition reduce + 8-way combine happens on host in float64.
"""

from contextlib import ExitStack

import numpy as np

import concourse.bass as bass
import concourse.mybir as mybir
import concourse.tile as tile
from concourse import bacc
from concourse.bass_utils import run_bass_kernel_spmd

# problem shape (hardcoded per harness contract)
N, C, H, W = 8, 19, 512, 1024
PIX = H * W          # 524288 pixels per core
P = 128              # partitions
FCOLS = PIX // P     # 4096 free columns when pixels laid out [128, 4096]
QW = FCOLS // 4      # 1024: quarter width
HQW = QW // 2        # 512: final-chunk / matmul width
NPAIR = C // 2       # 9 class pairs; class 18 is the unpaired tail class
N_CORES = 8
NSTAT = 16           # stats columns in the [128, 16] output

DT = mybir.dt

# stats column layout ([128, 16] f32; host folds):
#   0-2 : sum lnR      for quarters 0-2
#   3-4 : sum lnR      for quarter 3 chunks 0,1
#   8-10: sum pos*lnR  for quarters 0-2
#   11-12: sum pos*lnR for quarter 3 chunks 0,1
COL_LNR = 0
COL_POSLNR = 8

_ACT_TABLES_PATCHED = False


def _pin_act_table_set():
    """Restrict Ln/Exp to the natural_log_exp_and_others set so bacc's
    table-load pass emits a single ACT_TABLE_LOAD instead of thrashing
    between the ln-only and exp-only sets (~1.3us per reload).  Set
    indices must stay aligned with act_info.json, so every set entry is
    kept -- only the Ln/Exp membership of the other sets is dropped."""
    global _ACT_TABLES_PATCHED
    if _ACT_TABLES_PATCHED:
        return
    import concourse.bacc as bacc_mod

    orig = bacc_mod.get_activation_tables
    ln_exp = {mybir.ActivationFunctionType.Ln, mybir.ActivationFunctionType.Exp}

    def patched(arch):
        tables = orig(arch)
        return {
            name: (fns if name == "natural_log_exp_and_others" else fns - ln_exp)
            for name, fns in tables.items()
        }

    bacc_mod.get_activation_tables = patched
    _ACT_TABLES_PATCHED = True


def build_kernel() -> bass.Bass:
    _pin_act_table_set()

    # Bacc (not raw Bass): its compile() pipeline runs
    # generate_event_semaphores, which splits multi-sem waits to satisfy the
    # 1-wait-per-instruction TRN2 sync structs -- raw Bass modules with
    # Tile-emitted multi-waits fail walrus codegen.
    nc = bacc.Bacc("TRN2")

    predict = nc.declare_dram_parameter("predict", [C, PIX], DT.float32, isOutput=False)
    target = nc.declare_dram_parameter("target", [P, FCOLS], DT.bfloat16, isOutput=False)
    idn = nc.declare_dram_parameter("idn", [P, P], DT.bfloat16, isOutput=False)
    out = nc.declare_dram_parameter("out", [P, NSTAT], DT.float32, isOutput=True)

    pred_r = predict.rearrange("c (p f) -> c p f", p=P)  # [19, 128, 4096]
    pred_t = predict.rearrange("c (p f) -> p c f", p=P)  # [128, 19, 4096]

    with tile.TileContext(nc) as tc, ExitStack() as ctx:
        const = ctx.enter_context(tc.tile_pool(name="const", bufs=1))
        # p bufs=8 aligns slot reuse with the global DMA->DMAHW-proc
        # round-robin (8 procs), so the WAW on the old writer is same-proc
        # FIFO order and Tile emits no cross-queue wait
        p_pool = ctx.enter_context(tc.tile_pool(name="p", bufs=8))
        lm_pool = ctx.enter_context(tc.tile_pool(name="lm", bufs=4))
        eq_pool = ctx.enter_context(tc.tile_pool(name="eq", bufs=4))
        # class-18 (unpaired) tiles are smaller and only 1 per quarter
        p18_pool = ctx.enter_context(tc.tile_pool(name="p18", bufs=4))
        lm18_pool = ctx.enter_context(tc.tile_pool(name="lm18", bufs=2))
        eq18_pool = ctx.enter_context(tc.tile_pool(name="eq18", bufs=2))
        tail_pool = ctx.enter_context(tc.tile_pool(name="tail", bufs=2))
        psA_pool = ctx.enter_context(tc.tile_pool(name="psA", bufs=2, space="PSUM"))
        psL_pool = ctx.enter_context(tc.tile_pool(name="psL", bufs=2, space="PSUM"))

        t_bf = const.tile([P, FCOLS], DT.bfloat16, tag="tb")
        # quarter 0 of target first so the q0 eq chain is ready before p0
        nc.sync.dma_start(out=t_bf[:, 0:QW], in_=target[:, 0:QW])

        idn_sb = const.tile([P, P], DT.bfloat16, tag="idn")
        stats = const.tile([P, NSTAT], DT.float32, tag="stats")
        # per-partition -1.0 bias column for the fused Ln(expm - 1) tail
        negone = const.tile([P, 1], DT.float32, tag="negone")

        first_issued = False

        def post_first_dma():
            # small constants + remaining target quarters queue behind
            # p(q0,pair0) so the pipeline primes first
            nc.sync.dma_start(out=idn_sb[:], in_=idn[:])
            nc.vector.memset(stats[:], 0.0)
            nc.vector.memset(negone[:], -1.0)
            nc.sync.dma_start(out=t_bf[:, QW:], in_=target[:, QW:])

        for q in range(4):
            qbase = q * QW
            qsl_full = slice(qbase, qbase + QW)
            # PSUM accumulators for this quarter (ping-pong, 2+2 banks each)
            a_ps = psA_pool.tile([P, QW], DT.float32, tag="aps")
            l_ps = psL_pool.tile([P, QW], DT.float32, tag="lps")

            for pair in range(NPAIR):
                c = 2 * pair
                p_t = p_pool.tile([P, 2, QW], DT.float32, tag="p")
                nc.sync.dma_start(out=p_t[:, :, :], in_=pred_t[:, c : c + 2, qsl_full])

                if not first_issued:
                    first_issued = True
                    post_first_dma()

                # lm[:, 0:2, :] = L = Ln(1-p) bf16 ; lm[:, 2:4, :] = (T==c)*L
                lm = lm_pool.tile([P, 4, QW], DT.bfloat16, tag="lm")
                nc.scalar.activation(
                    out=lm[:, 0:2, :],
                    in_=p_t[:, :, :],
                    func=mybir.ActivationFunctionType.Ln,
                    bias=1.0,
                    scale=-1.0,
                )
                # eq at DVE 4x (16-bit tensor_scalar) + mult at 2x beats
                # the fused scalar_tensor_tensor, which only has a 1x uop
                eq = eq_pool.tile([P, 2, QW], DT.bfloat16, tag="eq")
                for j in range(2):
                    nc.vector.tensor_scalar(
                        out=eq[:, j, :],
                        in0=t_bf[:, qsl_full],
                        scalar1=float(c + j),
                        scalar2=None,
                        op0=mybir.AluOpType.is_equal,
                    )
                nc.vector.tensor_mul(
                    out=lm[:, 2:4, :],
                    in0=eq[:, 0:2, :],
                    in1=lm[:, 0:2, :],
                )

                # lsel matmuls first: l_ps frees early in the tail (Exp
                # is its only reader), so the next quarter's PE work
                # restarts sooner
                for mrow, which in ((2, "l"), (3, "l"), (0, "a"), (1, "a")):
                    dst_ps = l_ps if which == "l" else a_ps
                    cc = c + (mrow % 2)
                    for s in range(2):
                        nc.tensor.matmul(
                            dst_ps[:, s * HQW : (s + 1) * HQW],
                            lhsT=idn_sb[:],
                            rhs=lm[:, mrow, s * HQW : (s + 1) * HQW],
                            start=(cc == 0),
                            stop=False,
                        )

            # unpaired class 18; split into two 512 chunks in the last
            # quarter so the tail can start on chunk 0 while chunk 1
            # still computes
            chunks = [(s * HQW, HQW) for s in range(2)] if q == 3 else [(0, QW)]
            for off, width in chunks:
                csl = slice(qbase + off, qbase + off + width)
                p_s = p18_pool.tile([P, QW], DT.float32, tag="p18")
                nc.sync.dma_start(out=p_s[:, :width], in_=pred_r[C - 1, :, csl])
                lm = lm18_pool.tile([P, 2 * QW], DT.bfloat16, tag="lm18")
                nc.scalar.activation(
                    out=lm[:, :width],
                    in_=p_s[:, :width],
                    func=mybir.ActivationFunctionType.Ln,
                    bias=1.0,
                    scale=-1.0,
                )
                eq = eq18_pool.tile([P, QW], DT.bfloat16, tag="eq18")
                nc.vector.tensor_scalar(
                    out=eq[:, :width],
                    in0=t_bf[:, csl],
                    scalar1=float(C - 1),
                    scalar2=None,
                    op0=mybir.AluOpType.is_equal,
                )
                nc.vector.tensor_mul(
                    out=lm[:, QW : QW + width],
                    in0=eq[:, :width],
                    in1=lm[:, :width],
                )
                for s in range(width // HQW):
                    nc.tensor.matmul(
                        l_ps[:, off + s * HQW : off + (s + 1) * HQW],
                        lhsT=idn_sb[:],
                        rhs=lm[:, QW + s * HQW : QW + (s + 1) * HQW],
                        start=False,
                        stop=True,
                    )
                for s in range(width // HQW):
                    nc.tensor.matmul(
                        a_ps[:, off + s * HQW : off + (s + 1) * HQW],
                        lhsT=idn_sb[:],
                        rhs=lm[:, s * HQW : (s + 1) * HQW],
                        start=False,
                        stop=True,
                    )

            # tail: B = Ln(e^{-lsel} - 1) (bias fuses the -1); lnR = B + A.
            # quarter 3 drains in two 512 chunks to shorten the final latency
            # chain after the last DMA byte.
            tail_chunks = [(s * HQW, HQW) for s in range(2)] if q == 3 else [(0, QW)]
            for ci, (toff, twidth) in enumerate(tail_chunks):
                qsl = slice(toff, toff + twidth)
                col = q + ci if q < 3 else 3 + ci
                expm = tail_pool.tile([P, QW], DT.float32, tag="expm")
                nc.scalar.activation(
                    out=expm[:, :twidth],
                    in_=l_ps[:, qsl],
                    func=mybir.ActivationFunctionType.Exp,
                    scale=-1.0,
                )
                bb = tail_pool.tile([P, QW], DT.float32, tag="bb")
                nc.scalar.activation(
                    out=bb[:, :twidth],
                    in_=expm[:, :twidth],
                    func=mybir.ActivationFunctionType.Ln,
                    bias=negone[:],
                )
                lnr = tail_pool.tile([P, QW], DT.float32, tag="lnr")
                nc.vector.scalar_tensor_tensor(
                    out=lnr[:, :twidth],
                    in0=bb[:, :twidth],
                    scalar=0.0,
                    in1=a_ps[:, qsl],
                    op0=mybir.AluOpType.add,
                    op1=mybir.AluOpType.add,
                    accum_out=stats[:, COL_LNR + col : COL_LNR + col + 1],
                )
                scr = tail_pool.tile([P, QW], DT.float32, tag="scr")
                nc.vector.scalar_tensor_tensor(
                    out=scr[:, :twidth],
                    in0=t_bf[:, qbase + toff : qbase + toff + twidth],
                    scalar=0.5,
                    in1=lnr[:, :twidth],
                    op0=mybir.AluOpType.is_gt,
                    op1=mybir.AluOpType.mult,
                    accum_out=stats[:, COL_POSLNR + col : COL_POSLNR + col + 1],
                )

        nc.sync.dma_start(out=out[:], in_=stats[:])

    if not nc.is_finalized():
        nc.finalize()

    return nc


_NC_CACHE = None


def make_in_maps(predict: np.ndarray, target: np.ndarray):
    import ml_dtypes

    predict = np.ascontiguousarray(predict, dtype=np.float32)
    target_bf = np.ascontiguousarray(target, dtype=np.int32).astype(ml_dtypes.bfloat16)
    idn = np.eye(P, dtype=np.float32).astype(ml_dtypes.bfloat16)

    in_maps = []
    for k in range(N_CORES):
        in_maps.append(
            {
                "predict": predict[k].reshape(C, PIX),
                "target": target_bf[k].reshape(P, FCOLS),
                "idn": idn,
            }
        )
    return in_maps


def combine_host(results, target: np.ndarray) -> np.float32:
    tot = np.float64(0.0)
    s_all = np.float64(0.0)
    s_pos = np.float64(0.0)
    for k in range(N_CORES):
        st = results[k]["out"].reshape(P, NSTAT).astype(np.float64)
        s_all += -np.sum(st[:, COL_LNR : COL_LNR + 5])
        s_pos += -np.sum(st[:, COL_POSLNR : COL_POSLNR + 5])
        tot += PIX
    pos = np.float64(np.count_nonzero(target))
    neg = tot - pos
    s_neg = s_all - s_pos
    loss = ((neg / tot) * s_pos + (pos / tot) * s_neg) / (tot * C)
    return np.float32(loss)


def kernel(predict: np.ndarray, target: np.ndarray) -> np.ndarray:
    global _NC_CACHE
    if _NC_CACHE is None:
        _NC_CACHE = build_kernel()
    nc = _NC_CACHE

    in_maps = make_in_maps(predict, target)
    res = run_bass_kernel_spmd(nc, in_maps, list(range(N_CORES)))
    return combine_host(res.results, target)


# revision 10
# speedup vs baseline: 1.1252x; 1.1252x over previous
"""Weighted 2D cross-entropy (BCE-over-classes) loss on 8 Trainium2 cores.

Math (matches the reference):
  t in [0,19); pos = t>0, neg = t==0 (all pixels are pos or neg; mask == 1)
  S(i) = sum_c bce(i,c) = -lnR(i)
     lnR(i) = A(i) + B(i)
     A(i)   = sum_c ln(1-p_c(i))
     B(i)   = ln(p_t(i)) - ln(1-p_t(i)) = ln(e^{-lsel(i)} - 1),  lsel = ln(1-p_t)
  loss = ( (NEG/TOT)*S_pos_sum + (POS/TOT)*S_neg_sum ) / (TOT*C)

Per-core (core k <- batch element k, pure data parallel), FOUR quarter-passes
over pixel quarters [128, 1024].  A quarter's PSUM accumulators (A + lsel)
occupy 4 banks, so two quarters ping-pong in PSUM: while quarter q's tail
(Exp/Ln/STT chain on ACT+DVE) drains its PSUM banks, the PE already streams
quarter q+1's matmuls -- no half-boundary stall.
Classes are processed in PAIRS per quarter: one 3D DMA brings both classes'
quarter into a [128, 2, 1024] tile, so ACT runs ONE 2048-wide Ln and DVE ONE
2048-wide mult per pair -- the ~294ns fixed per-activation overhead halves
versus 1024-wide ops (ACT is the pipeline pacer once DMA stalls are gone).
  - per pair: 1MB DMA of p, ACT Ln(1-p)->bf16, DVE eq=(t==c) (4x, x2) and
    masked=eq*L (2x), PE identity-matmuls accumulate A and lsel in PSUM.
  - tail per quarter: expm=Exp(-lsel); B=Ln(expm-1) (fused -1 bias via a
    [128,1] const column, no DVE subtract); lnR=B+A via STT with accum_out;
    pos-masked sum via a second STT accum.
  - the unpaired class 18 runs per-quarter; in the last quarter it and the
    tail run in 512-wide chunks so the post-last-DMA drain is short.
Target is converted to bf16 on HOST (1MB instead of 2MB int32 DMA, no
on-chip CAST, and the first predict tile lands sooner).
Activation tables are pinned to natural_log_exp_and_others (holds both
ln and exp) -- otherwise bacc's table-load pass alternates between the
ln-only and exp-only sets, paying ~1.3us per reload.
Counts (pos/neg) are computed on host from the int target directly.
Per-core output is the raw [128, 16] per-partition stats; the final
partition reduce + 8-way combine happens on host in float64.
"""

from contextlib import ExitStack

import numpy as np

import concourse.bass as bass
import concourse.mybir as mybir
import concourse.tile as tile
from concourse import bacc
from concourse.bass_utils import run_bass_kernel_spmd

# problem shape (hardcoded per harness contract)
N, C, H, W = 8, 19, 512, 1024
PIX = H * W          # 524288 pixels per core
P = 128              # partitions
FCOLS = PIX // P     # 4096 free columns when pixels laid out [128, 4096]
QW = FCOLS // 4      # 1024: quarter width
HQW = QW // 2        # 512: final-chunk / matmul width
NPAIR = C // 2       # 9 class pairs; class 18 is the unpaired tail class
N_CORES = 8
NSTAT = 16           # stats columns in the [128, 16] output

DT = mybir.dt

# stats column layout ([128, 16] f32; host folds):
#   0-2 : sum lnR      for quarters 0-2
#   3-4 : sum lnR      for quarter 3 chunks 0,1
#   8-10: sum pos*lnR  for quarters 0-2
#   11-12: sum pos*lnR for quarter 3 chunks 0,1
COL_LNR = 0
COL_POSLNR = 8

_ACT_TABLES_PATCHED = False


def _pin_act_table_set():
    """Restrict Ln/Exp to the natural_log_exp_and_others set so bacc's
    table-load pass emits a single ACT_TABLE_LOAD instead of thrashing
    between the ln-only and exp-only sets (~1.3us per reload).  Set
    indices must stay aligned with act_info.json, so every set entry is
    kept -- only the Ln/Exp membership of the other sets is dropped."""
    global _ACT_TABLES_PATCHED
    if _ACT_TABLES_PATCHED:
        return
    import concourse.bacc as bacc_mod

    orig = bacc_mod.get_activation_tables
    ln_exp = {mybir.ActivationFunctionType.Ln, mybir.ActivationFunctionType.Exp}

    def patched(arch):
        tables = orig(arch)
        return {
            name: (fns if name == "natural_log_exp_and_others" else fns - ln_exp)
            for name, fns in tables.items()
        }

    bacc_mod.get_activation_tables = patched
    _ACT_TABLES_PATCHED = True


def build_kernel() -> bass.Bass:
    _pin_act_table_set()

    # Bacc (not raw Bass): its compile() pipeline runs
    # generate_event_semaphores, which splits multi-sem waits to satisfy the
    # 1-wait-per-instruction TRN2 sync structs -- raw Bass modules with
    # Tile-emitted multi-waits fail walrus codegen.
    nc = bacc.Bacc("TRN2")

    predict = nc.declare_dram_parameter("predict", [C, PIX], DT.float32, isOutput=False)
    target = nc.declare_dram_parameter("target", [P, FCOLS], DT.bfloat16, isOutput=False)
    idn = nc.declare_dram_parameter("idn", [P, P], DT.bfloat16, isOutput=False)
    out = nc.declare_dram_parameter("out", [P, NSTAT], DT.float32, isOutput=True)

    pred_r = predict.rearrange("c (p f) -> c p f", p=P)  # [19, 128, 4096]
    pred_t = predict.rearrange("c (p f) -> p c f", p=P)  # [128, 19, 4096]

    with tile.TileContext(nc) as tc, ExitStack() as ctx:
        const = ctx.enter_context(tc.tile_pool(name="const", bufs=1))
        # p bufs=8 aligns slot reuse with the global DMA->DMAHW-proc
        # round-robin (8 procs), so the WAW on the old writer is same-proc
        # FIFO order and Tile emits no cross-queue wait
        p_pool = ctx.enter_context(tc.tile_pool(name="p", bufs=8))
        lm_pool = ctx.enter_context(tc.tile_pool(name="lm", bufs=4))
        eq_pool = ctx.enter_context(tc.tile_pool(name="eq", bufs=4))
        # class-18 (unpaired) tiles are smaller and only 1 per quarter
        p18_pool = ctx.enter_context(tc.tile_pool(name="p18", bufs=4))
        lm18_pool = ctx.enter_context(tc.tile_pool(name="lm18", bufs=2))
        eq18_pool = ctx.enter_context(tc.tile_pool(name="eq18", bufs=2))
        tail_pool = ctx.enter_context(tc.tile_pool(name="tail", bufs=2))
        psA_pool = ctx.enter_context(tc.tile_pool(name="psA", bufs=2, space="PSUM"))
        psL_pool = ctx.enter_context(tc.tile_pool(name="psL", bufs=2, space="PSUM"))

        t_bf = const.tile([P, FCOLS], DT.bfloat16, tag="tb")
        # quarter 0 of target first so the q0 eq chain is ready before p0
        nc.sync.dma_start(out=t_bf[:, 0:QW], in_=target[:, 0:QW])

        idn_sb = const.tile([P, P], DT.bfloat16, tag="idn")
        stats = const.tile([P, NSTAT], DT.float32, tag="stats")
        # per-partition -1.0 bias column for the fused Ln(expm - 1) tail
        negone = const.tile([P, 1], DT.float32, tag="negone")

        first_issued = False

        def post_first_dma():
            # small constants + remaining target quarters queue behind
            # p(q0,pair0) so the pipeline primes first
            nc.sync.dma_start(out=idn_sb[:], in_=idn[:])
            nc.vector.memset(stats[:], 0.0)
            nc.vector.memset(negone[:], -1.0)
            nc.sync.dma_start(out=t_bf[:, QW:], in_=target[:, QW:])

        for q in range(4):
            qbase = q * QW
            qsl_full = slice(qbase, qbase + QW)
            # PSUM accumulators for this quarter (ping-pong, 2+2 banks each)
            a_ps = psA_pool.tile([P, QW], DT.float32, tag="aps")
            l_ps = psL_pool.tile([P, QW], DT.float32, tag="lps")

            for pair in range(NPAIR):
                c = 2 * pair
                # two plain 2D DMAs (128 descriptors each) into one tile:
                # a single 3D [p, c, f] DMA costs ~2.2us of serial
                # descriptor-generation on the sync sequencer vs ~0.6us x2
                p_t = p_pool.tile([P, 2, QW], DT.float32, tag="p")
                nc.sync.dma_start(out=p_t[:, 0, :], in_=pred_r[c, :, qsl_full])

                if not first_issued:
                    first_issued = True
                    post_first_dma()

                nc.sync.dma_start(out=p_t[:, 1, :], in_=pred_r[c + 1, :, qsl_full])

                # lm[:, 0:2, :] = L = Ln(1-p) bf16 ; lm[:, 2:4, :] = (T==c)*L
                lm = lm_pool.tile([P, 4, QW], DT.bfloat16, tag="lm")
                nc.scalar.activation(
                    out=lm[:, 0:2, :],
                    in_=p_t[:, :, :],
                    func=mybir.ActivationFunctionType.Ln,
                    bias=1.0,
                    scale=-1.0,
                )
                # eq at DVE 4x (16-bit tensor_scalar) + mult at 2x beats
                # the fused scalar_tensor_tensor, which only has a 1x uop
                eq = eq_pool.tile([P, 2, QW], DT.bfloat16, tag="eq")
                for j in range(2):
                    nc.vector.tensor_scalar(
                        out=eq[:, j, :],
                        in0=t_bf[:, qsl_full],
                        scalar1=float(c + j),
                        scalar2=None,
                        op0=mybir.AluOpType.is_equal,
                    )
                nc.vector.tensor_mul(
                    out=lm[:, 2:4, :],
                    in0=eq[:, 0:2, :],
                    in1=lm[:, 0:2, :],
                )

                # lsel matmuls first: l_ps frees early in the tail (Exp
                # is its only reader), so the next quarter's PE work
                # restarts sooner
                for mrow, which in ((2, "l"), (3, "l"), (0, "a"), (1, "a")):
                    dst_ps = l_ps if which == "l" else a_ps
                    cc = c + (mrow % 2)
                    for s in range(2):
                        nc.tensor.matmul(
                            dst_ps[:, s * HQW : (s + 1) * HQW],
                            lhsT=idn_sb[:],
                            rhs=lm[:, mrow, s * HQW : (s + 1) * HQW],
                            start=(cc == 0),
                            stop=False,
                        )

            # unpaired class 18; split into two 512 chunks in the last
            # quarter so the tail can start on chunk 0 while chunk 1
            # still computes
            chunks = [(s * HQW, HQW) for s in range(2)] if q == 3 else [(0, QW)]
            for off, width in chunks:
                csl = slice(qbase + off, qbase + off + width)
                p_s = p18_pool.tile([P, QW], DT.float32, tag="p18")
                nc.sync.dma_start(out=p_s[:, :width], in_=pred_r[C - 1, :, csl])
                lm = lm18_pool.tile([P, 2 * QW], DT.bfloat16, tag="lm18")
                nc.scalar.activation(
                    out=lm[:, :width],
                    in_=p_s[:, :width],
                    func=mybir.ActivationFunctionType.Ln,
                    bias=1.0,
                    scale=-1.0,
                )
                eq = eq18_pool.tile([P, QW], DT.bfloat16, tag="eq18")
                nc.vector.tensor_scalar(
                    out=eq[:, :width],
                    in0=t_bf[:, csl],
                    scalar1=float(C - 1),
                    scalar2=None,
                    op0=mybir.AluOpType.is_equal,
                )
                nc.vector.tensor_mul(
                    out=lm[:, QW : QW + width],
                    in0=eq[:, :width],
                    in1=lm[:, :width],
                )
                for s in range(width // HQW):
                    nc.tensor.matmul(
                        l_ps[:, off + s * HQW : off + (s + 1) * HQW],
                        lhsT=idn_sb[:],
                        rhs=lm[:, QW + s * HQW : QW + (s + 1) * HQW],
                        start=False,
                        stop=True,
                    )
                for s in range(width // HQW):
                    nc.tensor.matmul(
                        a_ps[:, off + s * HQW : off + (s + 1) * HQW],
                        lhsT=idn_sb[:],
                        rhs=lm[:, s * HQW : (s + 1) * HQW],
                        start=False,
                        stop=True,
                    )

            # tail: B = Ln(e^{-lsel} - 1) (bias fuses the -1); lnR = B + A.
            # quarter 3 drains in two 512 chunks to shorten the final latency
            # chain after the last DMA byte.
            tail_chunks = [(s * HQW, HQW) for s in range(2)] if q == 3 else [(0, QW)]
            for ci, (toff, twidth) in enumerate(tail_chunks):
                qsl = slice(toff, toff + twidth)
                col = q + ci if q < 3 else 3 + ci
                expm = tail_pool.tile([P, QW], DT.float32, tag="expm")
                nc.scalar.activation(
                    out=expm[:, :twidth],
                    in_=l_ps[:, qsl],
                    func=mybir.ActivationFunctionType.Exp,
                    scale=-1.0,
                )
                bb = tail_pool.tile([P, QW], DT.float32, tag="bb")
                nc.scalar.activation(
                    out=bb[:, :twidth],
                    in_=expm[:, :twidth],
                    func=mybir.ActivationFunctionType.Ln,
                    bias=negone[:],
                )
                lnr = tail_pool.tile([P, QW], DT.float32, tag="lnr")
                nc.vector.scalar_tensor_tensor(
                    out=lnr[:, :twidth],
                    in0=bb[:, :twidth],
                    scalar=0.0,
                    in1=a_ps[:, qsl],
                    op0=mybir.AluOpType.add,
                    op1=mybir.AluOpType.add,
                    accum_out=stats[:, COL_LNR + col : COL_LNR + col + 1],
                )
                scr = tail_pool.tile([P, QW], DT.float32, tag="scr")
                nc.vector.scalar_tensor_tensor(
                    out=scr[:, :twidth],
                    in0=t_bf[:, qbase + toff : qbase + toff + twidth],
                    scalar=0.5,
                    in1=lnr[:, :twidth],
                    op0=mybir.AluOpType.is_gt,
                    op1=mybir.AluOpType.mult,
                    accum_out=stats[:, COL_POSLNR + col : COL_POSLNR + col + 1],
                )

        nc.sync.dma_start(out=out[:], in_=stats[:])

    if not nc.is_finalized():
        nc.finalize()

    return nc


_NC_CACHE = None


def make_in_maps(predict: np.ndarray, target: np.ndarray):
    import ml_dtypes

    predict = np.ascontiguousarray(predict, dtype=np.float32)
    target_bf = np.ascontiguousarray(target, dtype=np.int32).astype(ml_dtypes.bfloat16)
    idn = np.eye(P, dtype=np.float32).astype(ml_dtypes.bfloat16)

    in_maps = []
    for k in range(N_CORES):
        in_maps.append(
            {
                "predict": predict[k].reshape(C, PIX),
                "target": target_bf[k].reshape(P, FCOLS),
                "idn": idn,
            }
        )
    return in_maps


def combine_host(results, target: np.ndarray) -> np.float32:
    tot = np.float64(0.0)
    s_all = np.float64(0.0)
    s_pos = np.float64(0.0)
    for k in range(N_CORES):
        st = results[k]["out"].reshape(P, NSTAT).astype(np.float64)
        s_all += -np.sum(st[:, COL_LNR : COL_LNR + 5])
        s_pos += -np.sum(st[:, COL_POSLNR : COL_POSLNR + 5])
        tot += PIX
    pos = np.float64(np.count_nonzero(target))
    neg = tot - pos
    s_neg = s_all - s_pos
    loss = ((neg / tot) * s_pos + (pos / tot) * s_neg) / (tot * C)
    return np.float32(loss)


def kernel(predict: np.ndarray, target: np.ndarray) -> np.ndarray:
    global _NC_CACHE
    if _NC_CACHE is None:
        _NC_CACHE = build_kernel()
    nc = _NC_CACHE

    in_maps = make_in_maps(predict, target)
    res = run_bass_kernel_spmd(nc, in_maps, list(range(N_CORES)))
    return combine_host(res.results, target)
